# revision 17
# baseline (speedup 1.0000x reference)
"""Trainium2 Bass kernel for nn_MultiHeadCrossAttention (BS=4, S=512, DM=512, H=8).

Sharding: one attention head per NeuronCore (8 heads / 8 cores). Each core
receives the full (transposed) q/k/v plus its head's weight slices, computes
its head end-to-end including the rank-64 slice of the output projection, and
the host sums the 8 partial outputs.

Math restructuring (validated against the reference numerically):
  E^T[kb][j,i] = exp(khT[kb]^T qhT[b]) computed per q-batch b in transposed
  layout; fenmu handled as W = 1/sum_kb E^T (the sqrt(DK)=8 factor is folded
  into Wv/bv on the host); RT = E^T[b] * W; score[i,(c,d)] = RT^T @ vh.
  softmax+LN over d uses: mean(sm) = 1/DK exactly; Sum(sm^2) = Q/Z^2 with
  Z = sum exp(score), Q = sum exp(2*score); std = exp(0.5*ln(63*var) +
  0.5*ln(1/63)); LN sum over c collapses to  sum_c e_c * w1_c  + w0, applied
  via per-partition tensor_scalar (gpsimd) and PE matmul accumulations.
"""

import numpy as np

BS, S, DM, H, DK = 4, 512, 512, 8, 64
EPS = 1e-6
NCORES = 8

F32 = None  # set lazily (mybir import)


def build_program(nc, tile, mybir):
    f32 = mybir.dt.float32
    f32r = mybir.dt.float32r
    bf16 = mybir.dt.bfloat16
    AF = mybir.ActivationFunctionType
    OP = mybir.AluOpType
    AX = mybir.AxisListType

    # ---- DRAM I/O ----
    qT_d = nc.dram_tensor("qT", [BS, DM, S], bf16, kind="ExternalInput")
    kT_d = nc.dram_tensor("kT", [BS, DM, S], bf16, kind="ExternalInput")
    vT_d = nc.dram_tensor("vT", [BS, DM, S], bf16, kind="ExternalInput")
    Wq_d = nc.dram_tensor("Wq", [DM, DK], bf16, kind="ExternalInput")
    Wk_d = nc.dram_tensor("Wk", [DM, DK], bf16, kind="ExternalInput")
    Wv_d = nc.dram_tensor("Wv", [DM, DK], bf16, kind="ExternalInput")  # pre-scaled 1/8
    bq_d = nc.dram_tensor("bq", [1, DK], bf16, kind="ExternalInput")
    bk_d = nc.dram_tensor("bk", [1, DK], bf16, kind="ExternalInput")
    bv_d = nc.dram_tensor("bv", [1, DK], bf16, kind="ExternalInput")  # pre-scaled 1/8
    Wo_d = nc.dram_tensor("Wo", [DK, DM], bf16, kind="ExternalInput")
    Wo4_d = nc.dram_tensor("Wo4", [DK, DM], bf16, kind="ExternalInput")  # 4*Wo_h
    bo_d = nc.dram_tensor("bo", [1, DM], bf16, kind="ExternalInput")  # zeros off core0
    al_d = nc.dram_tensor("alpha", [DK, 1], f32, kind="ExternalInput")
    b4_d = nc.dram_tensor("beta4", [DK, 1], f32, kind="ExternalInput")  # 4*beta
    id_d = nc.dram_tensor("ident", [128, 128], f32, kind="ExternalInput")
    outT_d = nc.dram_tensor("outT", [BS, DM, S], f32, kind="ExternalOutput")


    class _scope:
        def __init__(self, name):
            self.name = name
        def __enter__(self):
            self.sid, _ = nc.enter_named_scope(self.name, False)
        def __exit__(self, *a):
            nc.leave_named_scope(self.name, self.sid, False)

    with tile.TileContext(nc) as tc:
        with (
            tc.tile_pool(name="persist", bufs=1) as pp,
            tc.tile_pool(name="consts", bufs=1) as cp,
            tc.tile_pool(name="inp", bufs=3) as inp,
            tc.tile_pool(name="work", bufs=3) as wp,
            tc.tile_pool(name="vt", bufs=4) as vtp,
        ):
            # ---- persistent SBUF ----
            qhT = pp.tile([DK, BS, S], bf16, tag="qhT")
            khT = pp.tile([DK, BS, S], bf16, tag="khT")
            vh_all = pp.tile([128, 4, BS, DK], bf16, tag="vh")   # [j128, jc, c, d]
            rt_all = pp.tile([128, BS, 4, S], bf16, tag="rt")    # [j128, b, jc, i]
            e_all = pp.tile([128, 4, BS * BS * DK], f32, tag="e")  # [i128, ic, (b,c,d)]
            heads = pp.tile([DK, BS, S], bf16, tag="heads")        # [d, b, i]
            Z_all = pp.tile([128, 64], f32, tag="Z")
            Q_all = pp.tile([128, 64], f32, tag="Q")
            w1_all = pp.tile([128, 64], f32, tag="w1")
            w0_all = pp.tile([128, 16], f32, tag="w0")
            w0T = pp.tile([16, 128], f32, tag="w0T")
            w0f = pp.tile([1, 16 * 128], f32, tag="w0f")

            Wq_s = cp.tile([128, 4, DK], bf16, tag="Wq")
            Wk_s = cp.tile([128, 4, DK], bf16, tag="Wk")
            Wv_s = cp.tile([128, 4, DK], bf16, tag="Wv")
            Wo_s = cp.tile([DK, DM], bf16, tag="Wo")
            Wo4_s = cp.tile([DK, DM], bf16, tag="Wo4")
            bo_s = cp.tile([1, DM], bf16, tag="bo")
            bq_s = cp.tile([1, DK], bf16, tag="bq")
            bk_s = cp.tile([1, DK], bf16, tag="bk")
            bv_s = cp.tile([1, DK], bf16, tag="bv")
            al_s = cp.tile([DK, 1], f32, tag="al")
            b4_s = cp.tile([DK, 1], f32, tag="b4")
            id_s = cp.tile([128, 128], f32, tag="id")
            ones = cp.tile([1, S], bf16, tag="ones")
            ones_f = cp.tile([1, S], f32, tag="ones_f")
            bvb = cp.tile([128, DK], f32, tag="bvb")

            # ---- const loads ----
            nc.sync.dma_start(Wq_s[:], Wq_d.rearrange("(c p) d -> p c d", p=128))
            nc.sync.dma_start(Wk_s[:], Wk_d.rearrange("(c p) d -> p c d", p=128))
            nc.sync.dma_start(Wv_s[:], Wv_d.rearrange("(c p) d -> p c d", p=128))
            nc.sync.dma_start(Wo_s[:], Wo_d[:])
            nc.sync.dma_start(Wo4_s[:], Wo4_d[:])
            nc.sync.dma_start(bo_s[:], bo_d[:])
            nc.sync.dma_start(bq_s[:], bq_d[:])
            nc.sync.dma_start(bk_s[:], bk_d[:])
            nc.sync.dma_start(bv_s[:], bv_d[:])
            nc.sync.dma_start(al_s[:], al_d[:])
            nc.sync.dma_start(b4_s[:], b4_d[:])
            nc.sync.dma_start(id_s[:], id_d[:])
            nc.vector.memset(ones[:], 1.0)
            nc.vector.memset(ones_f[:], 1.0)

            vtiles = []
            with _scope("P1"), tc.tile_pool(name="ppsum", bufs=2, space="PSUM") as pps:
                # bv broadcast to 128 partitions: ones128^T (1x128) outer bv (1xDK)
                pb = pps.tile([128, DK], f32, tag="pbv")
                nc.tensor.matmul(pb[:], ones[:, 0:128], bv_s[:], start=True, stop=True)
                nc.vector.tensor_copy(bvb[:], pb[:])

                # ---- P1: k then q projections (khT/qhT gate P2) ----
                for W_s, b_s, dsrc, tag, dst in (
                    (Wk_s, bk_s, kT_d, "kt", khT), (Wq_s, bq_s, qT_d, "qt", qhT)
                ):
                    for b in range(BS):
                        src = inp.tile([128, 4, S], bf16, tag=tag)
                        nc.sync.dma_start(
                            src[:], dsrc[b].rearrange("(c p) s -> p c s", p=128))
                        ps = pps.tile([DK, S], f32, tag="proj")
                        for mc in range(4):
                            nc.tensor.matmul(
                                ps[:], W_s[:, mc, :], src[:, mc, :],
                                start=(mc == 0), stop=False,
                            )
                        nc.tensor.matmul(ps[:], b_s[:], ones[:],
                                         start=False, stop=True)
                        nc.scalar.activation(dst[:, b, :], ps[:], AF.Copy)

                # v loads early (DMA overlaps P2); projections deferred past P2
                for b in range(BS):
                    vt = vtp.tile([128, 4, S], bf16, tag=f"vt{b}")
                    nc.sync.dma_start(vt[:], vT_d[b].rearrange("(c p) s -> p c s", p=128))
                    vtiles.append(vt)

            # ---- P2: E^T, Ssum, RT per (b, jc) ----
            with _scope("P2"), tc.tile_pool(name="epsum", bufs=2, space="PSUM") as eps:
                for b in range(BS):
                    for jc in range(4):
                        pe = eps.tile([128, 4, S], f32, tag="pe")
                        for kb in range(4):
                            nc.tensor.matmul(
                                pe[:, kb, :],
                                khT[:, kb, jc * 128:(jc + 1) * 128],
                                qhT[:, b, :],
                                start=True, stop=True,
                            )
                        ex = wp.tile([128, 4, S], bf16, tag="ex")
                        nc.scalar.activation(ex[:], pe[:], AF.Exp)
                        t01 = wp.tile([128, S], bf16, tag="t01")
                        nc.gpsimd.tensor_tensor(t01[:], ex[:, 0, :], ex[:, 1, :], op=OP.add)
                        t23 = wp.tile([128, S], bf16, tag="t23")
                        nc.vector.tensor_tensor(t23[:], ex[:, 2, :], ex[:, 3, :], op=OP.add)
                        ssum = wp.tile([128, S], f32, tag="ssum")
                        nc.vector.tensor_tensor(ssum[:], t01[:], t23[:], op=OP.add)
                        wrec = wp.tile([128, S], f32, tag="wrec")
                        nc.vector.reciprocal_approx_fast(wrec[:], ssum[:])
                        nc.vector.tensor_tensor(
                            rt_all[:, b, jc, :], ex[:, b, :], wrec[:], op=OP.mult
                        )

            # ---- P1d: vh projections (v DMA overlapped P2; PE slots in now) ----
            with _scope("P1d"), tc.tile_pool(name="vpsum", bufs=2, space="PSUM") as vps:
                for b in range(BS):
                    vt = vtiles[b]
                    for jc in range(4):
                        pv = vps.tile([128, DK], f32, tag="pv")
                        for mc in range(4):
                            nc.tensor.matmul(
                                pv[:], vt[:, mc, jc * 128:(jc + 1) * 128],
                                Wv_s[:, mc, :],
                                start=(mc == 0), stop=(mc == 3),
                            )
                        nc.vector.tensor_tensor(
                            vh_all[:, jc, b, :], pv[:], bvb[:], op=mybir.AluOpType.add
                        )

            # ---- P3: score, exp, Z/Q ----
            with _scope("P3"), tc.tile_pool(name="spsum", bufs=2, space="PSUM") as sps:
                for ic in range(4):
                    pc = sps.tile([128, BS, BS * DK], f32, tag="pscore")
                    for b in range(BS):
                        for jc in range(4):
                            nc.tensor.matmul(
                                pc[:, b, :],
                                rt_all[:, b, jc, ic * 128:(ic + 1) * 128],
                                vh_all[:, jc].rearrange("p c d -> p (c d)"),
                                start=(jc == 0), stop=(jc == 3),
                            )
                    pcf = pc.rearrange("p b cd -> p (b cd)")
                    nc.scalar.activation(e_all[:, ic, :], pcf, AF.Exp)
                    e2 = wp.tile([128, BS * BS * DK], f32, tag="e2")
                    nc.scalar.activation(e2[:], pcf, AF.Exp, scale=2.0)
                    nc.vector.tensor_reduce(
                        Z_all[:, ic * 16:(ic + 1) * 16],
                        e_all[:, ic, :].rearrange("p (g d) -> p g d", d=DK),
                        axis=AX.X, op=OP.add,
                    )
                    nc.vector.tensor_reduce(
                        Q_all[:, ic * 16:(ic + 1) * 16],
                        e2[:].rearrange("p (g d) -> p g d", d=DK),
                        axis=AX.X, op=OP.add,
                    )

            # ---- P4: stats ----
            with _scope("P4"), tc.tile_pool(name="stats", bufs=1) as stp, \
                 tc.tile_pool(name="wpsum", bufs=1, space="PSUM") as wps:
                t = stp.tile([128, 64], f32, tag="t")
                nc.vector.tensor_tensor(t[:], Z_all[:], Z_all[:], op=OP.mult)  # Z^2
                t64 = stp.tile([128, 64], f32, tag="t64")
                nc.vector.tensor_scalar(t64[:], t[:], 1.0 / DK, None, op0=OP.mult)
                s = stp.tile([128, 64], f32, tag="s")
                nc.vector.tensor_tensor(s[:], Q_all[:], t64[:], op=OP.subtract)
                rinv = stp.tile([128, 64], f32, tag="rinv")
                nc.vector.reciprocal(rinv[:], t[:])
                v63 = stp.tile([128, 64], f32, tag="v63")
                nc.vector.tensor_tensor(v63[:], s[:], rinv[:], op=OP.mult)  # 63*var
                lnv = stp.tile([128, 64], f32, tag="lnv")
                nc.scalar.activation(lnv[:], v63[:], AF.Ln)
                lnbias = stp.tile([128, 1], f32, tag="lnbias")
                nc.vector.memset(lnbias[:], float(0.5 * np.log(1.0 / (DK - 1))))
                std = stp.tile([128, 64], f32, tag="std")
                nc.scalar.activation(
                    std[:], lnv[:], AF.Exp, scale=0.5, bias=lnbias[:],
                )
                stde = stp.tile([128, 64], f32, tag="stde")
                nc.vector.tensor_scalar(stde[:], std[:], EPS, None, op0=OP.add)
                g = stp.tile([128, 64], f32, tag="g")
                nc.vector.reciprocal(g[:], stde[:])
                zr = stp.tile([128, 64], f32, tag="zr")
                nc.vector.reciprocal(zr[:], Z_all[:])
                nc.vector.tensor_tensor(w1_all[:], g[:], zr[:], op=OP.mult)
                gs = stp.tile([128, 16], f32, tag="gs")
                nc.vector.tensor_reduce(
                    gs[:], g[:].rearrange("p (s c) -> p s c", c=4), axis=AX.X, op=OP.add
                )
                nc.vector.tensor_scalar(w0_all[:], gs[:], -1.0 / DK, None, op0=OP.mult)
                pw = wps.tile([16, 128], f32, tag="pw")
                nc.tensor.matmul(pw[:], w0_all[:], id_s[:], is_transpose=True,
                                 start=True, stop=True)
                nc.vector.tensor_copy(w0T[:], pw[:])
                nc.sync.dma_start(w0f[0:1, :].rearrange("o (s f) -> o s f", s=16),
                                  w0T[:, :])

            # ---- P5: B combine + transpose + alpha/beta ----
            with _scope("P5"), tc.tile_pool(name="bt", bufs=2, space="PSUM") as btp, \
                 tc.tile_pool(name="bwork", bufs=2) as bwp:
                for ic in range(4):
                    bsc = bwp.tile([128, BS, 4, DK], f32, tag="bsc")  # [i, b, c, d]
                    w1b = (w1_all[:, ic * 16:(ic + 1) * 16]
                           .rearrange("p (b c) -> p b c", c=4)
                           .unsqueeze(-1).broadcast_to((128, BS, 4, DK)))
                    nc.vector.tensor_tensor(
                        bsc[:],
                        e_all[:, ic, :].rearrange("p (b c d) -> p b c d", c=4, d=DK),
                        w1b, op=OP.mult,
                    )
                    ball = bwp.tile([128, BS, DK], f32, tag="ball")
                    nc.vector.tensor_reduce(
                        ball[:], bsc[:].rearrange("p b c d -> p b d c"),
                        axis=AX.X, op=OP.add,
                    )
                    for b in range(BS):
                        pbt = btp.tile([DK, 128], f32, tag="pbt")
                        nc.tensor.matmul(pbt[:], ball[:, b, :], id_s[:],
                                         is_transpose=True, start=True, stop=False)
                        slot = ic * 4 + b
                        nc.tensor.matmul(
                            pbt[:], ones_f[:, 0:DK],
                            w0f[0:1, slot * 128:(slot + 1) * 128],
                            start=False, stop=True,
                        )
                        nc.vector.tensor_scalar(
                            heads[:, b, ic * 128:(ic + 1) * 128], pbt[:],
                            al_s[:], b4_s[:], op0=OP.mult, op1=OP.add,
                        )

            # ---- P6: output projection ----
            with _scope("P6"), tc.tile_pool(name="opsum", bufs=2, space="PSUM") as ops, \
                 tc.tile_pool(name="owork", bufs=3) as owp:
                for b in range(BS):
                    for nch in range(4):
                        po = ops.tile([128, S], f32, tag="po")
                        nc.tensor.matmul(
                            po[:], Wo_s[:, nch * 128:(nch + 1) * 128],
                            heads[:, b, :], start=True, stop=False,
                        )
                        nc.tensor.matmul(
                            po[:], Wo4_s[:, nch * 128:(nch + 1) * 128],
                            qhT[:, b, :], start=False, stop=False,
                        )
                        nc.tensor.matmul(
                            po[:], bo_s[:, nch * 128:(nch + 1) * 128],
                            ones[:], start=False, stop=True,
                        )
                        ot = owp.tile([128, S], f32, tag="ot")
                        if (b + nch) % 2 == 0:
                            nc.scalar.activation(ot[:], po[:], AF.Copy)
                        else:
                            nc.vector.tensor_copy(ot[:], po[:])
                        nc.sync.dma_start(outT_d[b, nch * 128:(nch + 1) * 128, :], ot[:])

    nc._dbg_names = {
        "qhT": qhT.name, "khT": khT.name, "vh_all": vh_all.name,
        "rt_all": rt_all.name, "e_all": e_all.name, "heads": heads.name,
        "Z_all": Z_all.name, "Q_all": Q_all.name, "w1_all": w1_all.name,
        "w0_all": w0_all.name, "w0f": w0f.name,
    }
    return nc


def _build():
    import concourse.bass as bass  # noqa
    import concourse.tile as tile
    from concourse import bacc, mybir

    nc = bacc.Bacc("TRN2", target_bir_lowering=False, debug=False,
                   num_devices=NCORES)
    build_program(nc, tile, mybir)
    nc.compile()
    return nc


_cached_nc = None


def make_in_maps(q, k, v, Wq, bq, Wk, bk, Wv, bv, Wo, bo, alpha, beta):
    import ml_dtypes
    bft = ml_dtypes.bfloat16
    qT = np.ascontiguousarray(np.swapaxes(np.asarray(q, np.float32), 1, 2)).astype(bft)
    kT = np.ascontiguousarray(np.swapaxes(np.asarray(k, np.float32), 1, 2)).astype(bft)
    vT = np.ascontiguousarray(np.swapaxes(np.asarray(v, np.float32), 1, 2)).astype(bft)
    Wq, Wk, Wv, Wo = (np.asarray(x, np.float32) for x in (Wq, Wk, Wv, Wo))
    bq, bk, bv, bo = (np.asarray(x, np.float32) for x in (bq, bk, bv, bo))
    alpha, beta = np.asarray(alpha, np.float32), np.asarray(beta, np.float32)
    ident = np.eye(128, dtype=np.float32)
    scale = np.float32(1.0 / np.sqrt(np.float32(DK)))  # fenmu's sqrt(DK)=8, folded into Wv
    in_maps = []
    for h in range(NCORES):
        sl = slice(h * DK, (h + 1) * DK)
        in_maps.append({
            "qT": qT, "kT": kT, "vT": vT,
            "Wq": np.ascontiguousarray(Wq[:, sl]).astype(bft),
            "Wk": np.ascontiguousarray(Wk[:, sl]).astype(bft),
            "Wv": np.ascontiguousarray(Wv[:, sl] * scale).astype(bft),
            "bq": np.ascontiguousarray(bq[sl])[None, :].astype(bft),
            "bk": np.ascontiguousarray(bk[sl])[None, :].astype(bft),
            "bv": np.ascontiguousarray(bv[sl] * scale)[None, :].astype(bft),
            "Wo": np.ascontiguousarray(Wo[sl, :]).astype(bft),
            "Wo4": np.ascontiguousarray(4.0 * Wo[sl, :]).astype(bft),
            "bo": (bo if h == 0 else np.zeros_like(bo))[None, :].astype(bft),
            "alpha": np.ascontiguousarray(alpha)[:, None],
            "beta4": np.ascontiguousarray(4.0 * beta)[:, None],
            "ident": ident,
        })
    return in_maps


def assemble(results):
    out = np.zeros((BS, S, DM), np.float32)
    for r in results:
        out += np.swapaxes(r["outT"], 1, 2)
    return out


def kernel(**inputs) -> np.ndarray:
    global _cached_nc
    from concourse.bass_utils import run_bass_kernel_spmd

    if _cached_nc is None:
        _cached_nc = _build()
    in_maps = make_in_maps(**inputs)
    res = run_bass_kernel_spmd(_cached_nc, in_maps, list(range(NCORES)))
    return assemble(res.results)
